# revision 18
# baseline (speedup 1.0000x reference)
"""Distributed Trainium2 kernel for causal GQA attention with RoPE.

Model: B=2, S=2048, DM=2048, H=16 q-heads, HK=4 kv-heads, D=128.
Sharding over 8 NeuronCores: core c = (batch b=c//4, kv-head kh=c%4).
Each core computes its 4 q-heads / 1 kv-head of one batch end-to-end,
AllGathers attention outputs within its 4-core batch group, and applies
a column slice of Wo, producing out[b][:, kh*512:(kh+1)*512].

Schedule: proj chunks 0..3 back-to-back, then attention quarters in
order 3,2,1,0 (largest first).  One AllGather per quarter at its end.
Wo matmuls for quarter t are emitted as single-matmul "fillers"
interleaved into the score/PV loop of later quarters, so the PE never
starves while the exp chain (scalar engine) catches up.  Score blocks
are computed in [128,1024] PSUM pairs so one ACT covers two blocks.
Queue split: sync = x/wk loads + og loads + out stores, scalar queue =
wq/wv/wo loads, gpsimd = tables + rope shifts + broadcasts + cin stores
+ collectives.
"""
import contextlib
import ctypes
import os
import sys
import types
from collections import deque

for _p in ("/opt/trn_rl_repo", "/root/.axon_site/_ro/trn_rl_repo"):
    if os.path.isdir(_p) and _p not in sys.path:
        sys.path.insert(0, _p)

import numpy as np
import ml_dtypes

import concourse.bass as bass
import concourse.mybir as mybir
import concourse.tile as tile
from concourse import bacc
from concourse.bass import ts, ds
from concourse.bass_utils import run_bass_kernel_spmd

BF16 = ml_dtypes.bfloat16
F32 = mybir.dt.float32
BF = mybir.dt.bfloat16

B, S, DM = 2, 2048, 2048
H, HK, D = 16, 4, 128
G = H // HK          # q heads per kv head (= heads per core)
THETA = 10000.0
N_CORES = 8
KT = DM // 128       # 16 K-tiles of the model dim
TOKB = S // 128      # 16 token blocks
TCH = S // 512       # 4 token chunks of 512
HD_CORE = G * D      # 512 output dims of q per core
NEG = -1.0e30

LAST_EXEC_TIME_NS = None
LAST_RESULTS = None


# ---------------------------------------------------------------- tracing
def _install_ntff_hook():
    """Make run_bass_kernel_spmd(trace=True) work in this container."""
    try:
        from antenv.axon_hooks import get_axon_ntff_profile_hook  # noqa: F401
        return True
    except ImportError:
        pass
    so_path = "/opt/axon/libaxon_pjrt.so"
    if not os.path.exists(so_path):
        return False
    lib = ctypes.CDLL(so_path)
    if not hasattr(lib, "axon_start_nrt_profile"):
        return False
    lib.axon_start_nrt_profile.argtypes = [ctypes.POINTER(ctypes.c_int64), ctypes.c_size_t]
    lib.axon_start_nrt_profile.restype = ctypes.c_int64
    lib.axon_stop_nrt_profile.argtypes = [ctypes.c_char_p]
    lib.axon_stop_nrt_profile.restype = ctypes.c_int64

    @contextlib.contextmanager
    def _hook(output_dir, device_ids):
        import jax
        jax.devices()
        if device_ids:
            ids = (ctypes.c_int64 * len(device_ids))(*device_ids)
            rc = lib.axon_start_nrt_profile(ids, len(device_ids))
        else:
            rc = lib.axon_start_nrt_profile(None, 0)
        if rc != 0:
            raise RuntimeError(f"axon_start_nrt_profile rc={rc}")
        try:
            yield
        finally:
            n = lib.axon_stop_nrt_profile(str(output_dir).encode())
            print(f"profile: {n} file(s) in {output_dir}", file=sys.stderr)

    mod = types.ModuleType("antenv.axon_hooks")
    holder = {"h": _hook}
    mod.set_axon_ntff_profile_hook = lambda h: holder.__setitem__("h", h)
    mod.get_axon_ntff_profile_hook = lambda: holder.get("h")
    sys.modules["antenv.axon_hooks"] = mod
    import antenv
    antenv.axon_hooks = mod
    import concourse.bass_utils as bu
    bu.upload_artifacts = lambda tmpdir: str(tmpdir)
    return True


# ---------------------------------------------------------------- graph
def build_nc():
    nc = bacc.Bacc("TRN2", target_bir_lowering=False, debug=False,
                   num_devices=N_CORES)

    xt = nc.dram_tensor("xt", [DM, S], BF, kind="ExternalInput").ap()
    wq = nc.dram_tensor("wq", [DM, HD_CORE], BF, kind="ExternalInput").ap()
    wk = nc.dram_tensor("wk", [DM, D], BF, kind="ExternalInput").ap()
    wv = nc.dram_tensor("wv", [DM, D], BF, kind="ExternalInput").ap()
    wo = nc.dram_tensor("wo", [DM, HD_CORE], BF, kind="ExternalInput").ap()
    cosq = nc.dram_tensor("cosq", [D, S], F32, kind="ExternalInput").ap()
    sinq = nc.dram_tensor("sinq", [D, S], F32, kind="ExternalInput").ap()
    cosk = nc.dram_tensor("cosk", [D, S], F32, kind="ExternalInput").ap()
    sink = nc.dram_tensor("sink", [D, S], F32, kind="ExternalInput").ap()
    out = nc.dram_tensor("out", [S, HD_CORE], F32, kind="ExternalOutput").ap()

    groups = [[0, 1, 2, 3], [4, 5, 6, 7]]

    with tile.TileContext(nc) as tc:
        with tc.tile_pool(name="const", bufs=1) as cpool, \
             tc.tile_pool(name="wts", bufs=1) as wpool, \
             tc.tile_pool(name="acts", bufs=1) as apool, \
             tc.tile_pool(name="xin", bufs=32) as xpool, \
             tc.tile_pool(name="work", bufs=2) as work, \
             tc.tile_pool(name="etwork", bufs=4) as etwork, \
             tc.tile_pool(name="ogp", bufs=24) as ogpool, \
             tc.tile_pool(name="stats", bufs=4) as stats, \
             tc.tile_pool(name="bcp", bufs=2) as bcpool, \
             tc.tile_pool(name="psmm", bufs=2, space="PSUM") as ps_mm, \
             tc.tile_pool(name="pspv", bufs=2, space="PSUM") as ps_pv, \
             tc.tile_pool(name="psden", bufs=1, space="PSUM") as ps_den, \
             tc.tile_pool(name="pswo", bufs=1, space="PSUM") as ps_wo, \
             tc.tile_pool(name="dram", bufs=1, space="DRAM") as dpool:

            # ---------------- constants
            # transposed causal mask: keep [k_row p, q_col j] iff j >= p
            cmaskT = cpool.tile([128, 128], F32, tag="cmaskT", name="cmaskT")
            nc.gpsimd.memset(cmaskT[:], 0.0)
            nc.gpsimd.affine_select(
                out=cmaskT[:], in_=cmaskT[:],
                compare_op=mybir.AluOpType.is_ge, fill=NEG,
                base=0, pattern=[[1, 128]], channel_multiplier=-1)
            ones_sb = cpool.tile([128, 1], BF, tag="ones", name="ones")
            nc.gpsimd.memset(ones_sb[:], 1.0)
            # warm the ACT exp table so the first real exp is fast
            warm_act = cpool.tile([128, 1], F32, tag="warm_act",
                                  name="warm_act")
            nc.gpsimd.memset(warm_act[:], 0.0)
            nc.scalar.activation(out=warm_act[:], in_=warm_act[:],
                                 func=mybir.ActivationFunctionType.Exp)
            # warm up the collective path early (gpsimd-issued input DMA so
            # it doesn't queue behind the sync-engine load stream)
            warm_in = dpool.tile([128, 4], F32, tag="warm_in", name="warm_in")
            warm_out = dpool.tile([4, 128, 4], F32, tag="warm_out",
                                  name="warm_out")
            nc.gpsimd.dma_start(out=warm_in[:], in_=cosq[0:128, 0:4])
            nc.gpsimd.collective_compute(
                "AllGather", mybir.AluOpType.bypass,
                replica_groups=groups,
                ins=[warm_in.opt()], outs=[warm_out.opt()])

            # ---------------- weights + first x chunk, split across queues:
            # sync: x chunks + wk;  scalar: wq, wv, wo;  gpsimd: rope tables
            wq_sb = [wpool.tile([128, HD_CORE], BF, tag=f"wq{kt}",
                                name=f"wq{kt}") for kt in range(KT)]
            wk_sb = [wpool.tile([128, D], BF, tag=f"wk{kt}",
                                name=f"wk{kt}") for kt in range(KT)]
            wv_sb = [wpool.tile([128, D], BF, tag=f"wv{kt}",
                                name=f"wv{kt}") for kt in range(KT)]
            wo_sb = [wpool.tile([128, HD_CORE], BF, tag=f"wo{kt}",
                                name=f"wo{kt}") for kt in range(KT)]

            def load_xc(c):
                ts_ = [xpool.tile([128, 512], BF, tag="xc", name="xc")
                       for _ in range(KT)]
                for kt in range(KT):
                    nc.sync.dma_start(
                        out=ts_[kt][:],
                        in_=xt[ds(128 * kt, 128), ds(512 * c, 512)])
                return ts_

            for kt in range(KT):
                nc.scalar.dma_start(out=wk_sb[kt][:],
                                    in_=wk[ds(128 * kt, 128), :])
            xc_state = [load_xc(0)]
            for kt in range(KT):
                nc.scalar.dma_start(out=wq_sb[kt][:],
                                    in_=wq[ds(128 * kt, 128), :])
            tbl = {}
            for name, src in (("cosk", cosk), ("sink", sink),
                              ("cosq", cosq), ("sinq", sinq)):
                t = cpool.tile([D, S], F32, tag=name)
                nc.gpsimd.dma_start(out=t[:], in_=src[:])
                tbl[name] = t
            for kt in range(KT):
                nc.scalar.dma_start(out=wv_sb[kt][:],
                                    in_=wv[ds(128 * kt, 128), :])
            # wo is needed late (first Wo matmul ~150us in): put it on the
            # gpsimd queue behind the rope tables
            for kt in range(KT):
                nc.gpsimd.dma_start(out=wo_sb[kt][:],
                                    in_=wo[ds(128 * kt, 128), :])

            # ---------------- persistent activations
            qt_sb = [apool.tile([D, S], BF, tag=f"qt{h}", name=f"qt{h}")
                     for h in range(G)]
            kt_sb = apool.tile([D, S], BF, tag="kt", name="kt")
            vtok_sb = apool.tile([128, TOKB, D], BF, tag="vtok", name="vtok")

            # ---------------- projections + RoPE + direct token-major v
            def rope_store(raw_ps, c, dst_slice, cos_t, sin_t):
                # t2 = raw*cos straight from PSUM; the SBUF copy only feeds
                # the rotate-half shift (DMA cannot read PSUM)
                raw = work.tile([128, 512], F32, tag="qraw", name="qraw")
                nc.scalar.copy(raw[:], raw_ps)
                t2 = work.tile([128, 512], F32, tag="t2", name="t2")
                nc.vector.tensor_mul(t2[:], raw_ps,
                                     cos_t[:, ds(512 * c, 512)])
                sh = work.tile([128, 512], F32, tag="sh", name="sh")
                nc.gpsimd.dma_start(out=sh[0:64, :], in_=raw[64:128, :])
                nc.gpsimd.dma_start(out=sh[64:128, :], in_=raw[0:64, :])
                t1 = work.tile([128, 512], F32, tag="t1", name="t1")
                nc.vector.tensor_mul(t1[:], sh[:], sin_t[:, ds(512 * c, 512)])
                nc.vector.tensor_add(dst_slice, t1[:], t2[:])

            def emit_proj(c):
                xc = xc_state.pop(0)
                # pairs of projection groups share one [128,1024] PSUM tile
                # (2 banks): (k, q0), (q1, q2), (q3, v)
                ps = ps_mm.tile([128, 1024], F32, tag="mm", name="mm")
                for kt in range(KT):
                    nc.tensor.matmul(ps[:, 0:512], wk_sb[kt][:], xc[kt][:],
                                     start=(kt == 0), stop=(kt == KT - 1))
                rope_store(ps[:, 0:512], c, kt_sb[:, ds(512 * c, 512)],
                           tbl["cosk"], tbl["sink"])
                half = 1
                for h in range(G):
                    if half == 0:
                        ps = ps_mm.tile([128, 1024], F32, tag="mm", name="mm")
                    for kt in range(KT):
                        nc.tensor.matmul(ps[:, ds(512 * half, 512)],
                                         wq_sb[kt][:, ts(h, 128)], xc[kt][:],
                                         start=(kt == 0), stop=(kt == KT - 1))
                    rope_store(ps[:, ds(512 * half, 512)], c,
                               qt_sb[h][:, ds(512 * c, 512)],
                               tbl["cosq"], tbl["sinq"])
                    half ^= 1
                # v last, computed directly token-major: x-block stationary
                for tb in range(4):
                    for kt in range(KT):
                        nc.tensor.matmul(ps[:, ds(512 + 128 * tb, 128)],
                                         xc[kt][:, ts(tb, 128)], wv_sb[kt][:],
                                         start=(kt == 0), stop=(kt == KT - 1))
                nc.vector.tensor_copy(out=vtok_sb[:, ds(4 * c, 4), :],
                                      in_=ps[:, 512:1024])
                if c + 1 < TCH:
                    xc_state.append(load_xc(c + 1))

            # ---------------- collective buffers: one AllGather per quarter,
            # except quarter 0 (processed last) which splits into two
            # head-pair gathers so the tail gather hides under the Wo drain
            cin = [dpool.tile([D, G, 512], BF, tag=f"cin{t}", name=f"cin{t}")
                   for t in range(TCH)]
            cout = [dpool.tile([4, D, G, 512], BF, tag=f"cout{t}",
                               name=f"cout{t}") for t in range(TCH)]
            cin0 = [dpool.tile([D, 2, 512], BF, tag=f"cin0{p}",
                               name=f"cin0{p}") for p in range(2)]
            cout0 = [dpool.tile([4, D, 2, 512], BF, tag=f"cout0{p}",
                                name=f"cout0{p}") for p in range(2)]

            def cin_ap(qc, h):
                if qc == 0:
                    return cin0[h // 2][:, h % 2, :]
                return cin[qc][:, h, :]

            # ---------------- Wo: og loads + filler-granular matmuls
            og = {}

            def wo_loads(t):
                """Load all 16 gathered [D,512] tiles for quarter t into
                SBUF (sync queue; waits on the gather)."""
                ogs = [None] * KT
                # for the split quarter, emit pair-a loads before pair-b so
                # a pair-b load waiting on the second gather doesn't block
                # pair-a loads on the in-order sync queue
                order = sorted(range(KT), key=lambda kt: (kt % G >= 2, kt)) \
                    if t == 0 else range(KT)
                for kt in order:
                    r, h = divmod(kt, G)
                    o = ogpool.tile([128, 512], BF, tag="og", name="og")
                    if t == 0:
                        nc.sync.dma_start(out=o[:],
                                          in_=cout0[h // 2][r, :, h % 2, :])
                    else:
                        nc.sync.dma_start(out=o[:], in_=cout[t][r, :, h, :])
                    ogs[kt] = o
                og[t] = ogs

            pe_fill = deque()

            def queue_wo(t, pool=None, tag="wo"):
                """Enqueue quarter t's Wo work as single-matmul closures.
                During the final drain, pass pool=ps_pv tag="pv": attention
                is over, so its ring is free and this avoids single-buffer
                bubbles without extra PSUM banks."""
                pool = pool or ps_wo
                for tb in range(4):
                    state = {}

                    def mk(tb, idx, pool, state):
                        def f():
                            if idx == 0:
                                state["pw"] = pool.tile([128, 512], F32,
                                                        tag=tag, name="wo")
                            nc.tensor.matmul(state["pw"][:],
                                             og[t][idx][:, ts(tb, 128)],
                                             wo_sb[idx][:],
                                             start=(idx == 0),
                                             stop=(idx == KT - 1))
                            if idx == KT - 1:
                                ost = work.tile([128, 512], F32, tag="ost",
                                                name="ost")
                                nc.scalar.copy(ost[:], state["pw"][:])
                                nc.sync.dma_start(
                                    out=out[ds(512 * t + 128 * tb, 128), :],
                                    in_=ost[:])
                        return f

                    for idx in range(KT):
                        pe_fill.append(mk(tb, idx, pool, state))

            def pop_fill(n):
                for _ in range(n):
                    if not pe_fill:
                        return
                    pe_fill.popleft()()

            # ---------------- attention
            def emit_pair(h, qc, j):
                """Score pair (kb=2j, 2j+1), transposed [k 128, q 512] each,
                sharing one [128,1024] PSUM tile.  A DVE copy stages the
                scores to SBUF (bf16) so the PSUM tile frees ~1us after the
                matmuls instead of after the exp — decoupling the PE's score
                pipeline from ACT latency.  Returns (et2, offs[2])."""
                et2 = etwork.tile([128, 1024], BF, tag="et", name="et")
                sps = ps_mm.tile([128, 1024], F32, tag="mm", name="mm")
                stg = etwork.tile([128, 1024], BF, tag="stg", name="stg",
                                  bufs=3)
                offs = []
                for i in range(2):
                    kb = 2 * j + i
                    band = kb - 4 * qc
                    base = 512 * i
                    if band >= 0:
                        off = 128 * band
                        w = 512 - off
                        nc.tensor.matmul(sps[:, ds(base, w)],
                                         kt_sb[:, ts(kb, 128)],
                                         qt_sb[h][:, ds(512 * qc + off, w)],
                                         start=True, stop=True)
                        nc.vector.tensor_add(sps[:, ds(base, 128)],
                                             sps[:, ds(base, 128)], cmaskT[:])
                        offs.append(off)
                    else:
                        nc.tensor.matmul(sps[:, ds(base, 512)],
                                         kt_sb[:, ts(kb, 128)],
                                         qt_sb[h][:, ds(512 * qc, 512)],
                                         start=True, stop=True)
                        offs.append(0)
                nc.vector.tensor_copy(out=stg[:], in_=sps[:])
                if offs[0] == 0 and offs[1] == 0:
                    nc.scalar.activation(
                        out=et2[:], in_=stg[:],
                        func=mybir.ActivationFunctionType.Exp)
                else:
                    for i in range(2):
                        off, base = offs[i], 512 * i
                        if off:
                            nc.vector.memset(et2[:, ds(base, off)], 0.0)
                        # band matmul wrote scores left-aligned at base
                        nc.scalar.activation(
                            out=et2[:, ds(base + off, 512 - off)],
                            in_=stg[:, ds(base, 512 - off)],
                            func=mybir.ActivationFunctionType.Exp)
                return et2, offs

            def emit_attn(qc, enq):
                """enq: head -> list of actions, each ("loads", t) or
                ("wo", t), executed at the start of that head."""
                npair = 2 * qc + 2
                for h in range(G):
                    for act in enq.get(h, ()):
                        if act[0] == "loads":
                            wo_loads(act[1])
                        else:
                            queue_wo(act[1])
                    oT_ps = ps_pv.tile([128, 512], F32, tag="pv", name="pv")
                    den_ps = ps_den.tile([1, 512], F32, tag="den", name="den")
                    pend = [emit_pair(h, qc, 0)]
                    if npair > 1:
                        pend.append(emit_pair(h, qc, 1))
                    ngrp = (npair + 1) // 2
                    gsum = None
                    for j in range(npair):
                        # emit the lookahead scores BEFORE this pair's PV so
                        # the PE keeps feeding the exp pipeline while PV
                        # waits on exp j
                        if j + 2 < npair:
                            pend.append(emit_pair(h, qc, j + 2))
                        et2, offs = pend.pop(0)
                        for i in range(2):
                            kb, off, base = 2 * j + i, offs[i], 512 * i
                            nc.tensor.matmul(
                                oT_ps[:, ds(off, 512 - off)],
                                vtok_sb[:, kb, :],
                                et2[:, ds(base + off, 512 - off)],
                                start=(kb == 0),
                                stop=(kb == 2 * npair - 1))
                            pop_fill(2)
                        # pair-sum for the denominator (zeros in masked cols)
                        psum = etwork.tile([128, 512], BF, tag="psum",
                                           name="psum", bufs=3)
                        nc.vector.tensor_add(psum[:], et2[:, 0:512],
                                             et2[:, 512:1024])
                        if j % 2 == 0:
                            gsum = psum
                        if j % 2 == 1 or j == npair - 1:
                            if j % 2 == 1:
                                nsum = etwork.tile([128, 512], BF, tag="gsum",
                                                   name="gsum", bufs=2)
                                nc.vector.tensor_add(nsum[:], gsum[:],
                                                     psum[:])
                                gsum = nsum
                            nc.tensor.matmul(den_ps[:], ones_sb[:, 0:1],
                                             gsum[:],
                                             start=(j // 2 == 0),
                                             stop=(j // 2 == ngrp - 1))
                    rec = stats.tile([1, 512], F32, tag="recq", name="recq")
                    nc.vector.reciprocal_approx_fast(out=rec[:],
                                                     in_=den_ps[:])
                    bcast = bcpool.tile([128, 512], F32, tag="bcast",
                                        name="bcast")
                    nc.gpsimd.partition_broadcast(bcast[:], rec[:])
                    otst = work.tile([128, 512], BF, tag="otst", name="otst")
                    nc.vector.tensor_mul(otst[:], oT_ps[:], bcast[:])
                    nc.gpsimd.dma_start(out=cin_ap(qc, h), in_=otst[:])
                    if qc == 0 and h % 2 == 1:
                        p = h // 2
                        nc.gpsimd.collective_compute(
                            "AllGather", mybir.AluOpType.bypass,
                            replica_groups=groups,
                            ins=[cin0[p].opt()], outs=[cout0[p].opt()])
                    pop_fill(4)
                if qc != 0:
                    nc.gpsimd.collective_compute(
                        "AllGather", mybir.AluOpType.bypass,
                        replica_groups=groups,
                        ins=[cin[qc].opt()], outs=[cout[qc].opt()])

            # ---------------- schedule
            emit_proj(0)
            emit_proj(1)
            emit_proj(2)
            emit_proj(3)

            # Wo work for quarter t is enqueued a full quarter after its
            # gather fires, so a slow collective can never head-of-line
            # block the PE on an og-load wait.
            emit_attn(3, {})
            emit_attn(2, {0: [("loads", 3)]})
            emit_attn(1, {0: [("wo", 3), ("loads", 2)]})
            emit_attn(0, {0: [("wo", 2), ("loads", 1)]})
            pop_fill(len(pe_fill))
            queue_wo(1, pool=ps_pv, tag="pv")
            pop_fill(len(pe_fill))
            wo_loads(0)
            queue_wo(0, pool=ps_pv, tag="pv")
            pop_fill(len(pe_fill))

    nc.finalize()
    return nc


_NC_CACHE = {}


def _get_nc():
    if "nc" not in _NC_CACHE:
        _NC_CACHE["nc"] = build_nc()
    return _NC_CACHE["nc"]


def _rope_tables():
    inv = 1.0 / (THETA ** (np.arange(0, D, 2, dtype=np.float64) / D))  # [64]
    pos = np.arange(S, dtype=np.float64)
    fr = pos[:, None] * inv[None, :]                 # [S, 64]
    emb = np.concatenate([fr, fr], axis=1)           # [S, D]
    cos = np.cos(emb).T.astype(np.float32)           # [D, S]
    sin = np.sin(emb).T.astype(np.float32)
    sgn = np.where(np.arange(D) < D // 2, -1.0, 1.0).astype(np.float32)[:, None]
    scale = np.float32(D ** -0.5)
    return (cos * scale, sin * sgn * scale,          # q tables (pre-scaled)
            cos.copy(), sin * sgn)                   # k tables


def kernel(x, Wq, Wk, Wv, Wo):
    global LAST_EXEC_TIME_NS, LAST_RESULTS
    nc = _get_nc()
    cq, sq, ck, sk = _rope_tables()
    in_maps = []
    for c in range(N_CORES):
        b, kh = c // 4, c % 4
        in_maps.append({
            "xt": np.ascontiguousarray(x[b].T).astype(BF16),
            "wq": np.ascontiguousarray(Wq[:, kh * HD_CORE:(kh + 1) * HD_CORE]).astype(BF16),
            "wk": np.ascontiguousarray(Wk[:, kh * D:(kh + 1) * D]).astype(BF16),
            "wv": np.ascontiguousarray(Wv[:, kh * D:(kh + 1) * D]).astype(BF16),
            "wo": np.ascontiguousarray(Wo[:, kh * HD_CORE:(kh + 1) * HD_CORE]).astype(BF16),
            "cosq": cq, "sinq": sq, "cosk": ck, "sink": sk,
        })
    trace = os.environ.get("KERNEL_TRACE", "0") == "1" and _install_ntff_hook()
    if os.environ.get("KERNEL_WARMUP", "1") == "1":
        # Untraced warm-up execution: first-launch NEFF load/JIT skews the 8
        # cores by 10-100us, which lands in core 0's collective waits.  A
        # warm-up run aligns the cores so the measured run reflects the
        # kernel, not launch jitter.
        run_bass_kernel_spmd(nc, in_maps, core_ids=list(range(N_CORES)),
                             trace=False)
    res = run_bass_kernel_spmd(nc, in_maps, core_ids=list(range(N_CORES)),
                               trace=trace)
    LAST_EXEC_TIME_NS = res.exec_time_ns
    LAST_RESULTS = res
    out = np.empty((B, S, DM), dtype=np.float32)
    for c in range(N_CORES):
        b, kh = c // 4, c % 4
        out[b, :, kh * HD_CORE:(kh + 1) * HD_CORE] = res.results[c]["out"]
    return out


# revision 23
# speedup vs baseline: 1.2293x; 1.2293x over previous
"""Distributed Trainium2 kernel for causal GQA attention with RoPE.

Model: B=2, S=2048, DM=2048, H=16 q-heads, HK=4 kv-heads, D=128.
Sharding over 8 NeuronCores: core c = (batch b=c//4, kv-head kh=c%4).
Each core computes its 4 q-heads / 1 kv-head of one batch end-to-end,
AllGathers attention outputs within its 4-core batch group, and applies
a column slice of Wo, producing out[b][:, kh*512:(kh+1)*512].

Schedule: projection chunk 0, then attention quarters 0..3 with the
NEXT chunk's projection groups emitted between attention heads — the
dependency-free projection matmuls absorb the exp-chain (scalar ACT)
latency that otherwise stalls the PE inside a quarter.  Wo matmuls run
as single-matmul fillers popped inside later quarters' PV loops.  One
AllGather per quarter; the last quarter splits into two head-pair
gathers so the tail hides under the Wo drain.
Queues: sync = x + og loads + out stores; scalar = wk/wq/wv loads,
rope PSUM copies, exp, ost copies; gpsimd = tables + wo load + rope
shifts + broadcasts + cin stores + collectives.
"""
import contextlib
import ctypes
import os
import sys
import types
from collections import deque

for _p in ("/opt/trn_rl_repo", "/root/.axon_site/_ro/trn_rl_repo"):
    if os.path.isdir(_p) and _p not in sys.path:
        sys.path.insert(0, _p)

import numpy as np
import ml_dtypes

import concourse.bass as bass
import concourse.mybir as mybir
import concourse.tile as tile
from concourse import bacc
from concourse.bass import ts, ds
from concourse.bass_utils import run_bass_kernel_spmd

BF16 = ml_dtypes.bfloat16
F32 = mybir.dt.float32
BF = mybir.dt.bfloat16

B, S, DM = 2, 2048, 2048
H, HK, D = 16, 4, 128
G = H // HK          # q heads per kv head (= heads per core)
THETA = 10000.0
N_CORES = 8
KT = DM // 128       # 16 K-tiles of the model dim
TOKB = S // 128      # 16 token blocks
TCH = S // 512       # 4 token chunks of 512
HD_CORE = G * D      # 512 output dims of q per core
NEG = -1.0e30

LAST_EXEC_TIME_NS = None
LAST_RESULTS = None


# ---------------------------------------------------------------- tracing
def _install_ntff_hook():
    """Make run_bass_kernel_spmd(trace=True) work in this container."""
    try:
        from antenv.axon_hooks import get_axon_ntff_profile_hook  # noqa: F401
        return True
    except ImportError:
        pass
    so_path = "/opt/axon/libaxon_pjrt.so"
    if not os.path.exists(so_path):
        return False
    lib = ctypes.CDLL(so_path)
    if not hasattr(lib, "axon_start_nrt_profile"):
        return False
    lib.axon_start_nrt_profile.argtypes = [ctypes.POINTER(ctypes.c_int64), ctypes.c_size_t]
    lib.axon_start_nrt_profile.restype = ctypes.c_int64
    lib.axon_stop_nrt_profile.argtypes = [ctypes.c_char_p]
    lib.axon_stop_nrt_profile.restype = ctypes.c_int64

    @contextlib.contextmanager
    def _hook(output_dir, device_ids):
        import jax
        jax.devices()
        if device_ids:
            ids = (ctypes.c_int64 * len(device_ids))(*device_ids)
            rc = lib.axon_start_nrt_profile(ids, len(device_ids))
        else:
            rc = lib.axon_start_nrt_profile(None, 0)
        if rc != 0:
            raise RuntimeError(f"axon_start_nrt_profile rc={rc}")
        try:
            yield
        finally:
            n = lib.axon_stop_nrt_profile(str(output_dir).encode())
            print(f"profile: {n} file(s) in {output_dir}", file=sys.stderr)

    mod = types.ModuleType("antenv.axon_hooks")
    holder = {"h": _hook}
    mod.set_axon_ntff_profile_hook = lambda h: holder.__setitem__("h", h)
    mod.get_axon_ntff_profile_hook = lambda: holder.get("h")
    sys.modules["antenv.axon_hooks"] = mod
    import antenv
    antenv.axon_hooks = mod
    import concourse.bass_utils as bu
    bu.upload_artifacts = lambda tmpdir: str(tmpdir)
    return True


# ---------------------------------------------------------------- graph
def build_nc():
    nc = bacc.Bacc("TRN2", target_bir_lowering=False, debug=False,
                   num_devices=N_CORES)

    xt = nc.dram_tensor("xt", [DM, S], BF, kind="ExternalInput").ap()
    wq = nc.dram_tensor("wq", [DM, HD_CORE], BF, kind="ExternalInput").ap()
    wk = nc.dram_tensor("wk", [DM, D], BF, kind="ExternalInput").ap()
    wv = nc.dram_tensor("wv", [DM, D], BF, kind="ExternalInput").ap()
    wo = nc.dram_tensor("wo", [DM, HD_CORE], BF, kind="ExternalInput").ap()
    cosq = nc.dram_tensor("cosq", [D, S], F32, kind="ExternalInput").ap()
    sinq = nc.dram_tensor("sinq", [D, S], F32, kind="ExternalInput").ap()
    cosk = nc.dram_tensor("cosk", [D, S], F32, kind="ExternalInput").ap()
    sink = nc.dram_tensor("sink", [D, S], F32, kind="ExternalInput").ap()
    out = nc.dram_tensor("out", [S, HD_CORE], F32, kind="ExternalOutput").ap()

    groups = [[0, 1, 2, 3], [4, 5, 6, 7]]

    with tile.TileContext(nc) as tc:
        with tc.tile_pool(name="const", bufs=1) as cpool, \
             tc.tile_pool(name="wts", bufs=1) as wpool, \
             tc.tile_pool(name="acts", bufs=1) as apool, \
             tc.tile_pool(name="xin", bufs=64) as xpool, \
             tc.tile_pool(name="work", bufs=2) as work, \
             tc.tile_pool(name="etwork", bufs=7) as etwork, \
             tc.tile_pool(name="ogp", bufs=22) as ogpool, \
             tc.tile_pool(name="stats", bufs=2) as stats, \
             tc.tile_pool(name="bcp", bufs=2) as bcpool, \
             tc.tile_pool(name="psmm", bufs=4, space="PSUM") as ps_mm, \
             tc.tile_pool(name="pspv", bufs=2, space="PSUM") as ps_pv, \
             tc.tile_pool(name="psden", bufs=1, space="PSUM") as ps_den, \
             tc.tile_pool(name="pswo", bufs=1, space="PSUM") as ps_wo, \
             tc.tile_pool(name="dram", bufs=1, space="DRAM") as dpool:

            # ---------------- constants
            # transposed causal mask: keep [k_row p, q_col j] iff j >= p
            cmaskT = cpool.tile([128, 128], F32, tag="cmaskT", name="cmaskT")
            nc.gpsimd.memset(cmaskT[:], 0.0)
            nc.gpsimd.affine_select(
                out=cmaskT[:], in_=cmaskT[:],
                compare_op=mybir.AluOpType.is_ge, fill=NEG,
                base=0, pattern=[[1, 128]], channel_multiplier=-1)
            ones_sb = cpool.tile([128, 1], BF, tag="ones", name="ones")
            nc.gpsimd.memset(ones_sb[:], 1.0)
            # warm the ACT exp table so the first real exp is fast
            warm_act = cpool.tile([128, 1], F32, tag="warm_act",
                                  name="warm_act")
            nc.gpsimd.memset(warm_act[:], 0.0)
            nc.scalar.activation(out=warm_act[:], in_=warm_act[:],
                                 func=mybir.ActivationFunctionType.Exp)
            # warm up the collective path early (gpsimd-issued input DMA so
            # it doesn't queue behind the sync-engine load stream)
            warm_in = dpool.tile([128, 4], F32, tag="warm_in", name="warm_in")
            warm_out = dpool.tile([4, 128, 4], F32, tag="warm_out",
                                  name="warm_out")
            nc.gpsimd.dma_start(out=warm_in[:], in_=cosq[0:128, 0:4])
            nc.gpsimd.collective_compute(
                "AllGather", mybir.AluOpType.bypass,
                replica_groups=groups,
                ins=[warm_in.opt()], outs=[warm_out.opt()])

            # ---------------- loads: sync = x; scalar = wk, wq, wv;
            # gpsimd = rope tables + wo (needed last)
            wq_sb = [wpool.tile([128, HD_CORE], BF, tag=f"wq{kt}",
                                name=f"wq{kt}") for kt in range(KT)]
            wk_sb = [wpool.tile([128, D], BF, tag=f"wk{kt}",
                                name=f"wk{kt}") for kt in range(KT)]
            wv_sb = [wpool.tile([128, D], BF, tag=f"wv{kt}",
                                name=f"wv{kt}") for kt in range(KT)]
            wo_sb = [wpool.tile([128, HD_CORE], BF, tag=f"wo{kt}",
                                name=f"wo{kt}") for kt in range(KT)]

            def load_xc(c):
                ts_ = [xpool.tile([128, 512], BF, tag="xc", name="xc")
                       for _ in range(KT)]
                for kt in range(KT):
                    nc.sync.dma_start(
                        out=ts_[kt][:],
                        in_=xt[ds(128 * kt, 128), ds(512 * c, 512)])
                return ts_

            for kt in range(KT):
                nc.scalar.dma_start(out=wk_sb[kt][:],
                                    in_=wk[ds(128 * kt, 128), :])
            xc_state = [load_xc(0)]
            for kt in range(KT):
                nc.scalar.dma_start(out=wq_sb[kt][:],
                                    in_=wq[ds(128 * kt, 128), :])
            # q and k share one table pair; the D^-0.5 q-scale is folded into
            # the exp's free scale parameter instead
            tbl = {}
            for name, src in (("cosk", cosk), ("sink", sink)):
                t = cpool.tile([D, S], F32, tag=name)
                nc.gpsimd.dma_start(out=t[:], in_=src[:])
                tbl[name] = t
            for kt in range(KT):
                nc.scalar.dma_start(out=wv_sb[kt][:],
                                    in_=wv[ds(128 * kt, 128), :])
            # remaining x chunks up front: the sync queue blocks on gather
            # waits later (og loads), which must not delay x deliveries
            xc_state += [load_xc(c) for c in range(1, TCH)]
            for kt in range(KT):
                nc.gpsimd.dma_start(out=wo_sb[kt][:],
                                    in_=wo[ds(128 * kt, 128), :])

            # ---------------- persistent activations
            qt_sb = [apool.tile([D, S], BF, tag=f"qt{h}", name=f"qt{h}")
                     for h in range(G)]
            kt_sb = apool.tile([D, S], BF, tag="kt", name="kt")
            vtok_sb = apool.tile([128, TOKB, D], BF, tag="vtok", name="vtok")

            # ---------------- projections + RoPE + direct token-major v
            def rope_store(raw_ps, c, dst_slice, cos_t, sin_t):
                # t2 = raw*cos straight from PSUM; the SBUF copy only feeds
                # the rotate-half shift (DMA cannot read PSUM)
                raw = work.tile([128, 512], F32, tag="qraw", name="qraw")
                nc.scalar.copy(raw[:], raw_ps)
                t2 = work.tile([128, 512], F32, tag="t2", name="t2")
                nc.vector.tensor_mul(t2[:], raw_ps,
                                     cos_t[:, ds(512 * c, 512)])
                sh = work.tile([128, 512], F32, tag="sh", name="sh")
                nc.gpsimd.dma_start(out=sh[0:64, :], in_=raw[64:128, :])
                nc.gpsimd.dma_start(out=sh[64:128, :], in_=raw[0:64, :])
                t1 = work.tile([128, 512], F32, tag="t1", name="t1")
                nc.vector.tensor_mul(t1[:], sh[:], sin_t[:, ds(512 * c, 512)])
                nc.vector.tensor_add(dst_slice, t1[:], t2[:])

            def proj_groups(c):
                """Chunk c's projection as 6 thunks (k, q0..q3, v) to emit
                between attention heads."""
                st = {}

                def get_xc():
                    if "xc" not in st:
                        st["xc"] = xc_state.pop(0)
                    return st["xc"]

                def g_k():
                    xc = get_xc()
                    ps = ps_mm.tile([128, 512], F32, tag="mm", name="mm")
                    for kt in range(KT):
                        nc.tensor.matmul(ps[:], wk_sb[kt][:], xc[kt][:],
                                         start=(kt == 0), stop=(kt == KT - 1))
                    rope_store(ps[:], c, kt_sb[:, ds(512 * c, 512)],
                               tbl["cosk"], tbl["sink"])

                def mk_q(h):
                    def g_q():
                        xc = get_xc()
                        ps = ps_mm.tile([128, 512], F32, tag="mm", name="mm")
                        for kt in range(KT):
                            nc.tensor.matmul(ps[:], wq_sb[kt][:, ts(h, 128)],
                                             xc[kt][:],
                                             start=(kt == 0),
                                             stop=(kt == KT - 1))
                        rope_store(ps[:], c, qt_sb[h][:, ds(512 * c, 512)],
                                   tbl["cosk"], tbl["sink"])
                    return g_q

                def g_v():
                    xc = get_xc()
                    ps = ps_mm.tile([128, 512], F32, tag="mm", name="mm")
                    for tb in range(4):
                        for kt in range(KT):
                            nc.tensor.matmul(ps[:, ts(tb, 128)],
                                             xc[kt][:, ts(tb, 128)],
                                             wv_sb[kt][:],
                                             start=(kt == 0),
                                             stop=(kt == KT - 1))
                    nc.vector.tensor_copy(out=vtok_sb[:, ds(4 * c, 4), :],
                                          in_=ps[:])

                return [g_k, mk_q(0), mk_q(1), mk_q(2), mk_q(3), g_v]

            # ---------------- collective buffers: one AllGather per quarter,
            # except quarter 3 (processed last) which splits into two
            # head-pair gathers so the tail gather hides under the Wo drain
            cin = [dpool.tile([D, G, 512], BF, tag=f"cin{t}", name=f"cin{t}")
                   for t in range(TCH)]
            cout = [dpool.tile([4, D, G, 512], BF, tag=f"cout{t}",
                               name=f"cout{t}") for t in range(TCH)]
            cin3 = [dpool.tile([D, 2, 512], BF, tag=f"cin3{p}",
                               name=f"cin3{p}") for p in range(2)]
            cout3 = [dpool.tile([4, D, 2, 512], BF, tag=f"cout3{p}",
                                name=f"cout3{p}") for p in range(2)]

            def cin_ap(qc, h):
                if qc == 3:
                    return cin3[h // 2][:, h % 2, :]
                return cin[qc][:, h, :]

            # ---------------- Wo: og loads + filler-granular matmuls
            og = {}

            def wo_loads(t):
                """Load all 16 gathered [D,512] tiles for quarter t into
                SBUF (sync queue; waits on the gather)."""
                ogs = [None] * KT
                order = sorted(range(KT), key=lambda kt: (kt % G >= 2, kt)) \
                    if t == 3 else range(KT)
                for kt in order:
                    r, h = divmod(kt, G)
                    o = ogpool.tile([128, 512], BF, tag="og", name="og")
                    if t == 3:
                        nc.sync.dma_start(out=o[:],
                                          in_=cout3[h // 2][r, :, h % 2, :])
                    else:
                        nc.sync.dma_start(out=o[:], in_=cout[t][r, :, h, :])
                    ogs[kt] = o
                og[t] = ogs

            pe_fill = deque()

            def queue_wo(t, pool=None, tag="wo"):
                """Enqueue quarter t's Wo work as single-matmul closures."""
                pool = pool or ps_wo
                idx_order = sorted(range(KT), key=lambda i: (i % G >= 2, i)) \
                    if t == 3 else list(range(KT))
                for tb in range(4):
                    state = {}

                    def mk(tb, pos, idx, pool, state):
                        def f():
                            if pos == 0:
                                state["pw"] = pool.tile([128, 512], F32,
                                                        tag=tag, name="wo")
                            nc.tensor.matmul(state["pw"][:],
                                             og[t][idx][:, ts(tb, 128)],
                                             wo_sb[idx][:],
                                             start=(pos == 0),
                                             stop=(pos == KT - 1))
                            if pos == KT - 1:
                                ost = work.tile([128, 512], F32, tag="ost",
                                                name="ost")
                                nc.vector.tensor_copy(out=ost[:],
                                                      in_=state["pw"][:])
                                nc.sync.dma_start(
                                    out=out[ds(512 * t + 128 * tb, 128), :],
                                    in_=ost[:])
                        return f

                    for pos, idx in enumerate(idx_order):
                        pe_fill.append(mk(tb, pos, idx, pool, state))

            def pop_fill(n):
                for _ in range(n):
                    if not pe_fill:
                        return
                    pe_fill.popleft()()

            # ---------------- attention
            def emit_st(h, qc, kb):
                """score block, transposed: [k 128, q<=512] -> exp -> et"""
                band = kb - 4 * qc
                et = etwork.tile([128, 512], BF, tag="et", name="et")
                sps = ps_mm.tile([128, 512], F32, tag="mm", name="mm")
                if band >= 0:
                    off = 128 * band
                    w = 512 - off
                    nc.tensor.matmul(sps[:, :w], kt_sb[:, ts(kb, 128)],
                                     qt_sb[h][:, ds(512 * qc + off, w)],
                                     start=True, stop=True)
                    nc.vector.tensor_add(sps[:, :128], sps[:, :128], cmaskT[:])
                    if off:
                        nc.vector.memset(et[:, :off], 0.0)
                    nc.scalar.activation(
                        out=et[:, ds(off, w)], in_=sps[:, :w],
                        func=mybir.ActivationFunctionType.Exp,
                        scale=float(D) ** -0.5)
                    return et, off
                nc.tensor.matmul(sps[:], kt_sb[:, ts(kb, 128)],
                                 qt_sb[h][:, ds(512 * qc, 512)],
                                 start=True, stop=True)
                nc.scalar.activation(
                    out=et[:], in_=sps[:],
                    func=mybir.ActivationFunctionType.Exp,
                    scale=float(D) ** -0.5)
                return et, 0

            def emit_attn(qc, enq, post):
                """enq: head -> [("loads", t) | ("wo", t)] run at head start.
                post: head -> [thunks] (projection groups) run at head end."""
                for h in range(G):
                    for act in enq.get(h, ()):
                        if act[0] == "loads":
                            wo_loads(act[1])
                        else:
                            queue_wo(act[1])
                    nkb = 4 * qc + 4
                    oT_ps = ps_pv.tile([128, 512], F32, tag="pv", name="pv")
                    den_ps = ps_den.tile([1, 512], F32, tag="den", name="den")
                    pend = [emit_st(h, qc, k) for k in range(min(3, nkb))]
                    ngrp = (nkb + 3) // 4
                    esum = None
                    for kb in range(nkb):
                        if kb + 3 < nkb:
                            pend.append(emit_st(h, qc, kb + 3))
                        et, off = pend.pop(0)
                        nc.tensor.matmul(oT_ps[:, ds(off, 512 - off)],
                                         vtok_sb[:, kb, :],
                                         et[:, ds(off, 512 - off)],
                                         start=(kb == 0), stop=(kb == nkb - 1))
                        pop_fill(2)
                        # denominator: sum groups of 4 et tiles on DVE, then
                        # one ones-matmul per group
                        gi, gj = divmod(kb, 4)
                        last_in_grp = (gj == 3 or kb == nkb - 1)
                        if gj == 0:
                            esum = et
                        else:
                            nsum = etwork.tile([128, 512], BF, tag="esum",
                                               name="esum", bufs=3)
                            nc.vector.tensor_add(nsum[:], esum[:], et[:])
                            esum = nsum
                        if last_in_grp:
                            nc.tensor.matmul(den_ps[:], ones_sb[:, 0:1],
                                             esum[:],
                                             start=(gi == 0),
                                             stop=(gi == ngrp - 1))
                    rec = stats.tile([1, 512], F32, tag="recq", name="recq")
                    nc.vector.reciprocal_approx_fast(out=rec[:],
                                                     in_=den_ps[:])
                    bcast = bcpool.tile([128, 512], F32, tag="bcast",
                                        name="bcast")
                    nc.gpsimd.partition_broadcast(bcast[:], rec[:])
                    otst = work.tile([128, 512], BF, tag="otst", name="otst")
                    nc.vector.tensor_mul(otst[:], oT_ps[:], bcast[:])
                    nc.gpsimd.dma_start(out=cin_ap(qc, h), in_=otst[:])
                    if qc == 3 and h % 2 == 1:
                        p = h // 2
                        nc.gpsimd.collective_compute(
                            "AllGather", mybir.AluOpType.bypass,
                            replica_groups=groups,
                            ins=[cin3[p].opt()], outs=[cout3[p].opt()])
                    pop_fill(4)
                    for g in post.get(h, ()):
                        g()
                if qc != 3:
                    nc.gpsimd.collective_compute(
                        "AllGather", mybir.AluOpType.bypass,
                        replica_groups=groups,
                        ins=[cin[qc].opt()], outs=[cout[qc].opt()])

            # ---------------- schedule
            for g in proj_groups(0):
                g()
            pg = {c: proj_groups(c) for c in (1, 2, 3)}
            emit_attn(0, {},
                      {0: pg[1][0:2], 1: pg[1][2:4], 2: pg[1][4:6]})
            emit_attn(1, {0: [("loads", 0)], 3: [("wo", 0)]},
                      {0: pg[2][0:2], 1: pg[2][2:4], 2: pg[2][4:6]})
            emit_attn(2, {0: [("loads", 1)], 3: [("wo", 1)]},
                      {0: pg[3][0:2], 1: pg[3][2:4], 2: pg[3][4:6]})
            emit_attn(3, {0: [("loads", 2)], 3: [("wo", 2)]}, {})
            pop_fill(len(pe_fill))
            wo_loads(3)
            queue_wo(3, pool=ps_pv, tag="pv")
            pop_fill(len(pe_fill))

    nc.finalize()
    return nc


_NC_CACHE = {}


def _get_nc():
    if "nc" not in _NC_CACHE:
        _NC_CACHE["nc"] = build_nc()
    return _NC_CACHE["nc"]


def _rope_tables():
    inv = 1.0 / (THETA ** (np.arange(0, D, 2, dtype=np.float64) / D))  # [64]
    pos = np.arange(S, dtype=np.float64)
    fr = pos[:, None] * inv[None, :]                 # [S, 64]
    emb = np.concatenate([fr, fr], axis=1)           # [S, D]
    cos = np.cos(emb).T.astype(np.float32)           # [D, S]
    sin = np.sin(emb).T.astype(np.float32)
    sgn = np.where(np.arange(D) < D // 2, -1.0, 1.0).astype(np.float32)[:, None]
    scale = np.float32(D ** -0.5)
    return (cos * scale, sin * sgn * scale,          # q tables (pre-scaled)
            cos.copy(), sin * sgn)                   # k tables


def kernel(x, Wq, Wk, Wv, Wo):
    global LAST_EXEC_TIME_NS, LAST_RESULTS
    nc = _get_nc()
    cq, sq, ck, sk = _rope_tables()
    in_maps = []
    for c in range(N_CORES):
        b, kh = c // 4, c % 4
        in_maps.append({
            "xt": np.ascontiguousarray(x[b].T).astype(BF16),
            "wq": np.ascontiguousarray(Wq[:, kh * HD_CORE:(kh + 1) * HD_CORE]).astype(BF16),
            "wk": np.ascontiguousarray(Wk[:, kh * D:(kh + 1) * D]).astype(BF16),
            "wv": np.ascontiguousarray(Wv[:, kh * D:(kh + 1) * D]).astype(BF16),
            "wo": np.ascontiguousarray(Wo[:, kh * HD_CORE:(kh + 1) * HD_CORE]).astype(BF16),
            "cosq": cq, "sinq": sq, "cosk": ck, "sink": sk,
        })
    trace = os.environ.get("KERNEL_TRACE", "0") == "1" and _install_ntff_hook()
    if os.environ.get("KERNEL_WARMUP", "1") == "1":
        # Untraced warm-up execution: first-launch NEFF load/JIT skews the 8
        # cores by 10-100us, which lands in core 0's collective waits.  A
        # warm-up run aligns the cores so the measured run reflects the
        # kernel, not launch jitter.
        run_bass_kernel_spmd(nc, in_maps, core_ids=list(range(N_CORES)),
                             trace=False)
    res = run_bass_kernel_spmd(nc, in_maps, core_ids=list(range(N_CORES)),
                               trace=trace)
    LAST_EXEC_TIME_NS = res.exec_time_ns
    LAST_RESULTS = res
    out = np.empty((B, S, DM), dtype=np.float32)
    for c in range(N_CORES):
        b, kh = c // 4, c % 4
        out[b, :, kh * HD_CORE:(kh + 1) * HD_CORE] = res.results[c]["out"]
    return out
